# revision 1
# baseline (speedup 1.0000x reference)
"""MPNN layer on 8 Trainium2 NeuronCores (Bass/Tile).

Math (reference):
    m_edge = relu(x[dst] @ W1a^T + x[src] @ W1b^T + h @ W1c^T)        [E, D]
    m_node = segment_sum(m_edge, dst, N)                               [N, D]
    y      = m_node @ W2^T                                             [N, D]
    out_e  = relu(LN(snorm_e * y[src_e]))                              [E, D]
LN decomposition (exact):
    LN(s*v) = (v - mu_v) * s * rsqrt(s^2 * var_v + eps) * gamma + beta
so per-node stats (mu, var) are computed once per node and the per-edge part
is a scalar a_e = s_e * rsqrt(s_e^2 * var + eps) applied to the centered,
gamma-scaled node vector.

Sharding: edges partitioned by dst-bucket (node range) for phase 1 (each core
owns the complete segment-sum for its 1/8 of nodes - no reduction collective),
records (centered y + var) AllGathered, then phase 2 processes edges in
original order 1/8 chunks.

Segment-sum on PE: edges sorted by dst within a core; per 128-node block the
edge tiles matmul-accumulate (lhsT=m_edge tile [edge,feat], rhs=one-hot
[edge,node_rel]) into a psum [feat, node_rel]. One-hot built on DVE by
is_equal(iota_row, dst_rel); padded edge slots carry dst_rel=-1 giving a zero
one-hot column (exact zero contribution).

dma_gather uses int16 indices (<32768), so gathers from >32768-row tables are
split into a low call (rows [0,32768)) and a high call (rows [32768,...) with
indices rebased by -32768); edge slots are grouped [lo | hi] per block
(phase 1) / per shard (phase 2) so each call sees one range.
"""

import numpy as np
import ml_dtypes

from concourse import bacc, tile, mybir
from concourse import library_config
from concourse.bass_utils import run_bass_kernel_spmd

P = 128
LN_EPS = 1e-5
REC_W = 192            # record row: [yc(128) | var | pad..] f32; 768B (256B-mult)
BF16 = ml_dtypes.bfloat16

# ----------------------------------------------------------------------------
# host-side preprocessing
# ----------------------------------------------------------------------------


def _ceil_to(x, m):
    return -(-x // m) * m


def _wrap16(idx, dtype=np.int16):
    """[n] -> [128, n//16] int16: idx i at partition i%16, col i//16, replicated
    over the 8 groups of 16 partitions (each gpsimd q7 core reads its own 16)."""
    n = idx.shape[0]
    assert n % 16 == 0
    w = idx.reshape(n // 16, 16).T.astype(dtype)  # [16, n//16]
    return np.tile(w, (8, 1))


def _wrap128_cols(vals, n_tiles, fill, dtype=np.float32):
    """[n] -> [128, n_tiles]: value i at [i%128, i//128]; padded with fill."""
    out = np.full((n_tiles * P,), fill, dtype=dtype)
    out[: vals.shape[0]] = vals
    return out.reshape(n_tiles, P).T.copy()


class Plan:
    """All data-dependent layout decisions, computed on host from the inputs."""

    def __init__(self, n_nodes, n_edges, src, dst, nc=8, lo_limit=32768,
                 blk_nodes=128, p2_chunk_tiles=16):
        self.nc = nc
        self.n_nodes = n_nodes
        self.n_edges = n_edges
        self.lo_limit = lo_limit
        self.npc = n_nodes // nc                       # real nodes per core
        assert self.npc * nc == n_nodes
        self.npc_pad = _ceil_to(self.npc, blk_nodes)
        self.nblk = self.npc_pad // blk_nodes
        self.n_pad = self.npc_pad * nc                 # padded node table rows
        self.epc = n_edges // nc                       # phase-2 edges per core
        assert self.epc * nc == n_edges
        self.p2_chunk_tiles = p2_chunk_tiles

        src = np.asarray(src).astype(np.int64)
        dst = np.asarray(dst).astype(np.int64)
        self.src, self.dst = src, dst

        # ---- phase 1: bucket edges by dst core / block, lo/hi by src
        core_of = dst // self.npc
        blk_of = (dst - core_of * self.npc) // blk_nodes
        # mapped index of a node in slice-padded node tables (xb, records)
        self.node_map = (np.arange(n_nodes) // self.npc) * self.npc_pad + \
            (np.arange(n_nodes) % self.npc)
        is_lo1 = self.node_map[src] < lo_limit
        self.p1 = []       # per core: dict with per-block lo/hi edge id lists
        tl, th = 1, 0
        for c in range(nc):
            blocks = []
            in_c = core_of == c
            for b in range(self.nblk):
                m = in_c & (blk_of == b)
                lo_ids = np.nonzero(m & is_lo1)[0]
                hi_ids = np.nonzero(m & ~is_lo1)[0]
                blocks.append((lo_ids, hi_ids))
                tl = max(tl, -(-len(lo_ids) // P))
                th = max(th, -(-len(hi_ids) // P))
            self.p1.append(blocks)
        self.tl, self.th = tl, th
        self.t_blk = tl + th                            # tiles per block
        self.t1 = self.nblk * self.t_blk                # phase-1 tiles per core
        self.e1 = self.t1 * P

        # ---- phase 2: original-order shards, lo/hi by mapped src
        self.mapped_src = self.node_map[src]
        lo2_max, hi2_max = 1, 0
        self.p2 = []
        for c in range(nc):
            ids = np.arange(c * self.epc, (c + 1) * self.epc)
            m = self.mapped_src[ids] < lo_limit
            lo_ids, hi_ids = ids[m], ids[~m]
            self.p2.append((lo_ids, hi_ids))
            lo2_max = max(lo2_max, len(lo_ids))
            hi2_max = max(hi2_max, len(hi_ids))
        ct = p2_chunk_tiles * P
        self.lo2 = _ceil_to(lo2_max, ct) // P           # tiles in lo region
        self.hi2 = _ceil_to(hi2_max, ct) // P
        self.t2 = self.lo2 + self.hi2
        self.e2 = self.t2 * P

    # ---- per-core input arrays -------------------------------------------
    def core_inputs(self, c, x, h, snorm_n, W1, W2):
        p = self
        f32, i16 = np.float32, np.int16

        # phase-1 slot -> edge id (-1 for pad)
        slots = np.full(p.e1, -1, dtype=np.int64)
        for b, (lo_ids, hi_ids) in enumerate(p.p1[c]):
            base = b * p.t_blk * P
            slots[base: base + len(lo_ids)] = lo_ids
            base += p.tl * P
            slots[base: base + len(hi_ids)] = hi_ids
        pad = slots < 0
        e_ids = np.where(pad, 0, slots)

        h_t = np.ascontiguousarray(h[e_ids].T).astype(BF16)
        h_t[:, pad] = BF16(0.0)

        dst_loc = self.dst[e_ids] - c * p.npc
        dst_rel = dst_loc - (np.arange(p.e1) // (p.t_blk * P)) * 128
        dst_rel = np.where(pad, -1.0, dst_rel.astype(f32))
        dst_rel_w = dst_rel.reshape(p.t1, P).T.copy().astype(f32)  # [128, t1]

        idx_xa = np.where(pad, 0, dst_loc).astype(np.int64)
        src1 = np.where(pad, 0, self.node_map[self.src[e_ids]])
        # hi slots: rebase by lo_limit (pads in hi region -> 0)
        in_hi = np.zeros(p.e1, dtype=bool)
        for b in range(p.nblk):
            s = b * p.t_blk * P + p.tl * P
            in_hi[s: s + p.th * P] = True
        idx_xb = np.where(in_hi, np.maximum(src1 - p.lo_limit, 0), src1)
        idx_xb = np.where(pad, 0, idx_xb)

        # phase 2
        lo_ids, hi_ids = p.p2[c]
        slots2 = np.full(p.e2, -1, dtype=np.int64)
        slots2[: len(lo_ids)] = lo_ids
        slots2[p.lo2 * P: p.lo2 * P + len(hi_ids)] = hi_ids
        pad2 = slots2 < 0
        e2_ids = np.where(pad2, 0, slots2)
        mapped = self.mapped_src[e2_ids]
        idx_rec = np.where(np.arange(p.e2) >= p.lo2 * P,
                           np.maximum(mapped - p.lo_limit, 0), mapped)
        idx_rec = np.where(pad2, 0, idx_rec)
        sn = snorm_n.reshape(-1)[e2_ids].astype(f32)
        sn = np.where(pad2, 1.0, sn)

        return {
            "h_t": h_t,
            "dst_rel": dst_rel_w,
            "idx_xb": _wrap16(idx_xb),
            "idx_rec": _wrap16(idx_rec),
            "snorm": _wrap128_cols(sn, p.t2, 1.0),
        }, slots2


# ----------------------------------------------------------------------------
# bass program
# ----------------------------------------------------------------------------


def build_program(p: Plan, use_gamma: bool, use_beta: bool, stage="full"):
    # stage in {tables, phase1, ag, full} - debug bisect: later stages omitted

    dt = mybir.dt
    nc = bacc.Bacc(None)
    nc.gpsimd.load_library(library_config.mlp)

    n_xt = p.n_pad                  # node table rows (x_t cols)
    lo_rows = min(p.lo_limit, n_xt)
    hi_rows = n_xt - lo_rows

    # ---- parameters (per-core shapes; replicated arrays passed identically)
    x_t = nc.declare_dram_parameter("x_t", [P, n_xt], dt.bfloat16, isOutput=False)
    x_tl = nc.declare_dram_parameter("x_tl", [P, p.npc_pad], dt.bfloat16, isOutput=False)
    h_t = nc.declare_dram_parameter("h_t", [P, p.e1], dt.bfloat16, isOutput=False)
    w1aT = nc.declare_dram_parameter("w1aT", [P, P], dt.bfloat16, isOutput=False)
    w1bT = nc.declare_dram_parameter("w1bT", [P, P], dt.bfloat16, isOutput=False)
    w1cT = nc.declare_dram_parameter("w1cT", [P, P], dt.bfloat16, isOutput=False)
    w2T = nc.declare_dram_parameter("w2T", [P, P], dt.bfloat16, isOutput=False)
    ident_in = nc.declare_dram_parameter("ident", [P, P], dt.bfloat16, isOutput=False)
    iota_in = nc.declare_dram_parameter("iota", [P, P], dt.float32, isOutput=False)
    dst_rel = nc.declare_dram_parameter("dst_rel", [P, p.t1], dt.float32, isOutput=False)
    idx_xb = nc.declare_dram_parameter("idx_xb", [P, p.e1 // 16], dt.int16, isOutput=False)
    idx_rec = nc.declare_dram_parameter("idx_rec", [P, p.e2 // 16], dt.int16, isOutput=False)
    snorm = nc.declare_dram_parameter("snorm", [P, p.t2], dt.float32, isOutput=False)
    gamma_b = beta_b = None
    if use_gamma:
        gamma_b = nc.declare_dram_parameter("gamma_b", [P, P], dt.float32, isOutput=False)
    if use_beta:
        beta_b = nc.declare_dram_parameter("beta_b", [P, P], dt.float32, isOutput=False)

    out = nc.declare_dram_parameter("out", [p.e2, P], dt.float32, isOutput=True)

    # ---- internal DRAM
    xb_dram = nc.dram_tensor("xb_dram", [n_xt, P], dt.float32)
    rec_local = nc.dram_tensor("rec_local", [p.npc_pad, REC_W], dt.float32)
    rec_addr_space = "Shared" if p.nc > 4 else "Local"
    rec_full = nc.dram_tensor("rec_full", [p.n_pad, REC_W], dt.float32,
                              addr_space=rec_addr_space)

    f32, bf16 = dt.float32, dt.bfloat16
    GMAX = 8    # dma_gather is limited to 1024 indices (8 tiles) per call

    def gather_tiles(out_tile, in_ap, idx_sb, slot0, n_tiles, elem, tile_off=0):
        for g0 in range(0, n_tiles, GMAX):
            gn = min(GMAX, n_tiles - g0)
            e0 = slot0 + g0 * P
            nc.gpsimd.dma_gather(
                out_ap=out_tile[:, tile_off + g0: tile_off + g0 + gn, :],
                in_ap=in_ap,
                idxs_ap=idx_sb[:, e0 // 16: (e0 + gn * P) // 16],
                num_idxs=gn * P, num_idxs_reg=gn * P, elem_size=elem)

    with tile.TileContext(nc) as tc:
        with tc.tile_pool(name="const", bufs=1) as cpool, \
             tc.tile_pool(name="xtile", bufs=3) as xpool, \
             tc.tile_pool(name="tabout", bufs=3) as tpool, \
             tc.tile_pool(name="blk", bufs=2) as bpool, \
             tc.tile_pool(name="edge", bufs=3) as epool, \
             tc.tile_pool(name="nodeep", bufs=2) as npool, \
             tc.tile_pool(name="p2", bufs=2) as p2pool, \
             tc.tile_pool(name="psA", bufs=2, space="PSUM") as psA, \
             tc.tile_pool(name="psT", bufs=2, space="PSUM") as psT, \
             tc.tile_pool(name="psSeg", bufs=2, space="PSUM") as psSeg, \
             tc.tile_pool(name="psY", bufs=2, space="PSUM") as psY:

            # ---- constants
            w1aT_sb = cpool.tile([P, P], bf16, tag="w1a")
            w1bT_sb = cpool.tile([P, P], bf16, tag="w1b")
            w1cT_sb = cpool.tile([P, P], bf16, tag="w1c")
            w2T_sb = cpool.tile([P, P], bf16, tag="w2")
            ident_sb = cpool.tile([P, P], bf16, tag="ident")
            iota_sb = cpool.tile([P, P], f32, tag="iota")
            dstrel_sb = cpool.tile([P, p.t1], f32, tag="dstrel")
            ixb_sb = cpool.tile([P, p.e1 // 16], dt.int16, tag="ixb")
            irec_sb = cpool.tile([P, p.e2 // 16], dt.int16, tag="irec")
            snorm_sb = cpool.tile([P, p.t2], f32, tag="snorm")
            eps_sb = cpool.tile([P, 1], f32, tag="eps")
            nc.vector.memset(eps_sb[:], LN_EPS)
            nc.sync.dma_start(out=w1aT_sb[:], in_=w1aT[:])
            nc.sync.dma_start(out=w1bT_sb[:], in_=w1bT[:])
            nc.sync.dma_start(out=w1cT_sb[:], in_=w1cT[:])
            nc.sync.dma_start(out=w2T_sb[:], in_=w2T[:])
            nc.sync.dma_start(out=ident_sb[:], in_=ident_in[:])
            nc.sync.dma_start(out=iota_sb[:], in_=iota_in[:])
            nc.sync.dma_start(out=dstrel_sb[:], in_=dst_rel[:])
            nc.sync.dma_start(out=ixb_sb[:], in_=idx_xb[:])
            nc.sync.dma_start(out=irec_sb[:], in_=idx_rec[:])
            nc.sync.dma_start(out=snorm_sb[:], in_=snorm[:])
            gamma_sb = beta_sb = None
            if use_gamma:
                gamma_sb = cpool.tile([P, P], f32, tag="gam")
                nc.sync.dma_start(out=gamma_sb[:], in_=gamma_b[:])
            if use_beta:
                beta_sb = cpool.tile([P, P], f32, tag="bet")
                nc.sync.dma_start(out=beta_sb[:], in_=beta_b[:])

            # ---- node tables: xa (core slice), xb (all nodes)
            def table_mm(x_src, col0, w_sb, dram, row0):
                xt = xpool.tile([P, P], bf16, tag="xt")
                nc.sync.dma_start(out=xt[:], in_=x_src[:, col0:col0 + P])
                ps = psA.tile([P, P], f32, tag="psm")
                nc.tensor.matmul(out=ps[:], lhsT=xt[:], rhs=w_sb[:],
                                 start=True, stop=True)
                t = tpool.tile([P, P], f32, tag="tabout")
                nc.vector.tensor_copy(out=t[:], in_=ps[:])
                nc.sync.dma_start(out=dram[row0:row0 + P, :], in_=t[:])

            for j in range(n_xt // P):
                table_mm(x_t, j * P, w1bT_sb, xb_dram, j * P)

            # ---- phase 1 + 1.5, per block
            inv_d = 1.0 / P
            for b in (range(p.nblk) if stage not in ("tables",) else []):
                base_t = b * p.t_blk          # first tile of block
                base_e = base_t * P

                h_sb = bpool.tile([P, p.t_blk * P], bf16, tag="hblk")
                nc.sync.dma_start(out=h_sb[:],
                                  in_=h_t[:, base_e: base_e + p.t_blk * P])

                # xa for this block's 128 dst nodes: computed on PE, kept in
                # SBUF; per-edge selection happens via the transposed one-hot.
                xt_b = xpool.tile([P, P], bf16, tag="xtb")
                nc.sync.dma_start(out=xt_b[:], in_=x_tl[:, b * P:(b + 1) * P])
                ps_xa = psY.tile([P, P], f32, tag="psy")
                nc.tensor.matmul(out=ps_xa[:], lhsT=xt_b[:], rhs=w1aT_sb[:],
                                 start=True, stop=True)
                xa_sb = bpool.tile([P, P], bf16, tag="xasb")
                nc.scalar.copy(out=xa_sb[:], in_=ps_xa[:])

                xb_g = bpool.tile([P, p.t_blk, P], f32, tag="xbg")
                gather_tiles(xb_g, xb_dram[:lo_rows, :], ixb_sb, base_e, p.tl, P)
                if p.th > 0 and hi_rows > 0:
                    gather_tiles(xb_g, xb_dram[lo_rows:, :], ixb_sb,
                                 base_e + p.tl * P, p.th, P, tile_off=p.tl)

                if stage == "p1load":
                    continue
                ps_seg = psSeg.tile([P, P], f32, tag="seg")
                for tt in range(p.t_blk):
                    oh = epool.tile([P, P], bf16, tag="oh")
                    t_glob = base_t + tt
                    nc.vector.tensor_scalar(
                        out=oh[:], in0=iota_sb[:],
                        scalar1=dstrel_sb[:, t_glob:t_glob + 1], scalar2=None,
                        op0=mybir.AluOpType.is_equal)
                    ps_t = psT.tile([P, P], bf16, tag="pst")
                    nc.tensor.transpose(ps_t[:], oh[:], ident_sb[:])
                    ohT = epool.tile([P, P], bf16, tag="ohT")
                    nc.scalar.copy(out=ohT[:], in_=ps_t[:])
                    ps_m = psA.tile([P, P], f32, tag="psm")
                    nc.tensor.matmul(out=ps_m[:],
                                     lhsT=h_sb[:, tt * P:(tt + 1) * P],
                                     rhs=w1cT_sb[:], start=True, stop=False)
                    nc.tensor.matmul(out=ps_m[:], lhsT=ohT[:], rhs=xa_sb[:],
                                     start=False, stop=True)
                    tsum2 = epool.tile([P, P], f32, tag="tsum2")
                    nc.vector.tensor_tensor(out=tsum2[:], in0=xb_g[:, tt, :],
                                            in1=ps_m[:], op=mybir.AluOpType.add)
                    me = epool.tile([P, P], bf16, tag="me")
                    nc.scalar.activation(
                        out=me[:], in_=tsum2[:],
                        func=mybir.ActivationFunctionType.Relu)
                    nc.tensor.matmul(out=ps_seg[:], lhsT=me[:], rhs=oh[:],
                                     start=(tt == 0), stop=(tt == p.t_blk - 1))

                if stage == "p1mm":
                    continue
                # phase 1.5: y, stats, record
                mnT = npool.tile([P, P], bf16, tag="mnT")
                nc.vector.tensor_copy(out=mnT[:], in_=ps_seg[:])
                ps_y = psY.tile([P, P], f32, tag="psy")
                nc.tensor.matmul(out=ps_y[:], lhsT=mnT[:], rhs=w2T_sb[:],
                                 start=True, stop=True)

                rec = npool.tile([P, REC_W], f32, tag="rec")
                nc.vector.memset(rec[:], 0.0)
                if stage == "p1y":
                    nc.vector.tensor_copy(out=rec[:, 0:P], in_=ps_y[:])
                    nc.sync.dma_start(out=rec_local[b * P:(b + 1) * P, :],
                                      in_=rec[:])
                    continue
                mu = npool.tile([P, 1], f32, tag="mu")
                nc.vector.tensor_reduce(out=mu[:], in_=ps_y[:],
                                        axis=mybir.AxisListType.X,
                                        op=mybir.AluOpType.add)
                nc.vector.tensor_scalar_mul(mu[:], mu[:], inv_d)
                nc.vector.tensor_scalar(
                    out=rec[:, 0:P], in0=ps_y[:], scalar1=mu[:], scalar2=None,
                    op0=mybir.AluOpType.subtract)
                if stage == "p1stats":
                    nc.sync.dma_start(out=rec_local[b * P:(b + 1) * P, :],
                                      in_=rec[:])
                    continue
                sq = npool.tile([P, P], f32, tag="sq")
                nc.vector.tensor_tensor(out=sq[:], in0=rec[:, 0:P],
                                        in1=rec[:, 0:P],
                                        op=mybir.AluOpType.mult)
                vsum = npool.tile([P, 1], f32, tag="vsum")
                nc.vector.tensor_reduce(out=vsum[:], in_=sq[:],
                                        axis=mybir.AxisListType.X,
                                        op=mybir.AluOpType.add)
                nc.vector.tensor_scalar_mul(rec[:, P:P + 1], vsum[:], inv_d)
                if use_gamma:
                    nc.vector.tensor_tensor(out=rec[:, 0:P], in0=rec[:, 0:P],
                                            in1=gamma_sb[:],
                                            op=mybir.AluOpType.mult)
                nc.sync.dma_start(out=rec_local[b * P:(b + 1) * P, :],
                                  in_=rec[:])

            # ---- AllGather records
            if stage in ("ag", "full"):
                nc.gpsimd.collective_compute(
                "AllGather", mybir.AluOpType.bypass,
                    replica_groups=[list(range(p.nc))],
                    ins=[rec_local[:]], outs=[rec_full[:]])

            # ---- phase 2
            ct = p.p2_chunk_tiles
            n_chunks = p.t2 // ct if stage == "full" else 0
            rec_lo_rows = min(p.lo_limit, p.n_pad)
            for ch in range(n_chunks):
                t0 = ch * ct
                e0 = t0 * P
                is_hi = t0 >= p.lo2
                rec_g = p2pool.tile([P, ct, REC_W], f32, tag="recg")
                src_ap = rec_full[rec_lo_rows:, :] if is_hi else \
                    rec_full[:rec_lo_rows, :]
                gather_tiles(rec_g, src_ap, irec_sb, e0, ct, REC_W)

                # batched per-chunk LN scale: a_e = s*rsqrt(s^2*var + eps)
                sn_ap = snorm_sb[:, t0:t0 + ct]
                s2 = p2pool.tile([P, ct], f32, tag="s2")
                nc.vector.tensor_tensor(out=s2[:], in0=sn_ap, in1=sn_ap,
                                        op=mybir.AluOpType.mult)
                q2 = p2pool.tile([P, ct], f32, tag="q2")
                nc.vector.tensor_tensor(out=q2[:], in0=rec_g[:, :, P:P + 1],
                                        in1=s2[:], op=mybir.AluOpType.mult)
                q = p2pool.tile([P, ct], f32, tag="q")
                nc.scalar.activation(out=q[:], in_=q2[:],
                                     func=mybir.ActivationFunctionType.Sqrt,
                                     bias=eps_sb[:])
                rq = p2pool.tile([P, ct], f32, tag="rq")
                nc.vector.reciprocal(out=rq[:], in_=q[:])
                a = p2pool.tile([P, ct], f32, tag="a")
                nc.vector.tensor_tensor(out=a[:], in0=rq[:], in1=sn_ap,
                                        op=mybir.AluOpType.mult)

                out_sb = p2pool.tile([P, ct, P], f32, tag="outsb")
                for tt in range(ct):
                    if use_beta:
                        t1 = p2pool.tile([P, P], f32, tag="t1")
                        nc.vector.tensor_scalar(
                            out=t1[:], in0=rec_g[:, tt, 0:P],
                            scalar1=a[:, tt:tt + 1],
                            scalar2=None, op0=mybir.AluOpType.mult)
                        nc.vector.tensor_tensor(out=t1[:], in0=t1[:],
                                                in1=beta_sb[:],
                                                op=mybir.AluOpType.add)
                        nc.scalar.activation(
                            out=out_sb[:, tt, :], in_=t1[:],
                            func=mybir.ActivationFunctionType.Relu)
                    else:
                        nc.scalar.activation(
                            out=out_sb[:, tt, :], in_=rec_g[:, tt, 0:P],
                            func=mybir.ActivationFunctionType.Relu,
                            scale=a[:, tt:tt + 1])

                out_view = out[e0: e0 + ct * P, :].rearrange(
                    "(t p) d -> p t d", p=P)
                nc.sync.dma_start(out=out_view, in_=out_sb[:])

    nc.finalize()
    return nc


# ----------------------------------------------------------------------------
# driver
# ----------------------------------------------------------------------------


def _prep_inputs(p: Plan, x, h, snorm_n, W1, W2, ln_gamma, ln_beta):
    D = P
    use_gamma = not np.allclose(ln_gamma, 1.0)
    use_beta = not np.allclose(ln_beta, 0.0)

    x_t_full = np.zeros((D, p.n_pad), dtype=BF16)
    # x.T laid out per-core-slice: table row (c*npc_pad + i) = node c*npc + i
    xt = np.asarray(x).T.astype(BF16)
    for c in range(p.nc):
        x_t_full[:, c * p.npc_pad: c * p.npc_pad + p.npc] = \
            xt[:, c * p.npc: (c + 1) * p.npc]

    common = {
        "x_t": x_t_full,
        "w1aT": np.ascontiguousarray(W1[:, :D].T).astype(BF16),
        "w1bT": np.ascontiguousarray(W1[:, D:2 * D].T).astype(BF16),
        "w1cT": np.ascontiguousarray(W1[:, 2 * D:3 * D].T).astype(BF16),
        "w2T": np.ascontiguousarray(W2.T).astype(BF16),
        "ident": np.eye(P, dtype=np.float32).astype(BF16),
        "iota": np.tile(np.arange(P, dtype=np.float32), (P, 1)),
    }
    if use_gamma:
        common["gamma_b"] = np.tile(np.asarray(ln_gamma, np.float32), (P, 1))
    if use_beta:
        common["beta_b"] = np.tile(np.asarray(ln_beta, np.float32), (P, 1))

    in_maps, slots2_all = [], []
    for c in range(p.nc):
        m, slots2 = p.core_inputs(c, x, h, snorm_n, W1, W2)
        m.update(common)
        m["x_tl"] = np.ascontiguousarray(
            x_t_full[:, c * p.npc_pad: (c + 1) * p.npc_pad])
        in_maps.append(m)
        slots2_all.append(slots2)
    return in_maps, slots2_all, use_gamma, use_beta


def run(x, h, snorm_n, W1, W2, ln_gamma, ln_beta, src, dst, n_cores=8,
        lo_limit=32768, trace=False, stage="full"):
    n_nodes, n_edges = x.shape[0], h.shape[0]
    p = Plan(n_nodes, n_edges, src, dst, nc=n_cores, lo_limit=lo_limit)
    in_maps, slots2_all, use_gamma, use_beta = _prep_inputs(
        p, x, h, snorm_n, W1, W2, ln_gamma, ln_beta)
    nc = build_program(p, use_gamma, use_beta, stage=stage)
    res = run_bass_kernel_spmd(nc, in_maps, core_ids=list(range(n_cores)),
                               trace=trace)
    out = np.empty((n_edges, P), dtype=np.float32)
    for c in range(n_cores):
        o = res.results[c]["out"]
        s = slots2_all[c]
        real = s >= 0
        out[s[real]] = o[real]
    return out, res


def kernel(x, h, snorm_n, snorm_e, W1, W2, ln_gamma, ln_beta, src, dst):
    out, _ = run(np.asarray(x), np.asarray(h), np.asarray(snorm_n),
                 np.asarray(W1), np.asarray(W2), np.asarray(ln_gamma),
                 np.asarray(ln_beta), np.asarray(src), np.asarray(dst))
    return out



# revision 13
# speedup vs baseline: 1.8338x; 1.8338x over previous
"""MPNN layer on 8 Trainium2 NeuronCores (Bass/Tile) - v2, collective-free.

Math (reference):
    m_edge = relu(x[dst] @ W1a^T + x[src] @ W1b^T + h @ W1c^T)        [E, D]
    m_node = segment_sum(m_edge, dst, N)                               [N, D]
    y      = m_node @ W2^T                                             [N, D]
    out_e  = relu(LN(snorm_n_e * y[src_e]))                            [E, D]

LN decomposition (exact, s>0):
    LN(s*v)*gamma+beta = (v - mu_v)*gamma * a_e + beta,
    a_e = s*rsqrt(s^2*var_v + eps) = rsqrt(var_v + eps/s^2)
and for beta==0:  relu(yc*gamma * a_e) = a_e * relu(yc*gamma)  (a_e > 0),
so the per-node record stores relu'd centered y and var; the per-edge part is
a scalar multiply.

Sharding (no collectives, each core independent):
  phase 1: edges bucketed by dst-owner core + 128-node dst block. Segment-sum
    per block via one-hot matmuls. x[src] fetched by transpose dma_gather of
    the node-major bf16 x table (global); x[dst] selected on PE via one-hot;
    h term via per-tile matmul.
  phase 1.5: per block: y = m@W2, mean/var/relu-center; records stay in SBUF.
  phase 2: edges bucketed by src-owner core + src block (every core owns the
    records it needs). One-hot select of record + var on PE; a = rsqrt(...)
    computed on batched psum row-vectors; output written bf16 in bucketed
    order; host inverts the permutation.
"""

import numpy as np
import ml_dtypes

from concourse import bacc, tile, mybir
from concourse import library_config
from concourse.bass_utils import run_bass_kernel_spmd

P = 128
LN_EPS = 1e-5
BF16 = ml_dtypes.bfloat16

# ----------------------------------------------------------------------------
# host-side planning
# ----------------------------------------------------------------------------


def _ceil_to(x, m):
    return -(-x // m) * m


def _wrap16(idx, dtype=np.int16):
    """[n] -> [128, n//16] int16: idx i at partition i%16, col i//16, replicated
    over the 8 groups of 16 partitions."""
    n = idx.shape[0]
    assert n % 16 == 0
    w = idx.reshape(n // 16, 16).T.astype(dtype)
    return np.tile(w, (8, 1))


class Plan:
    def __init__(self, n_nodes, n_edges, src, dst, nc=8, lo_limit=32768):
        self.nc = nc
        self.n_nodes = n_nodes
        self.n_edges = n_edges
        self.lo_limit = lo_limit
        self.npc = n_nodes // nc
        assert self.npc * nc == n_nodes
        self.npc_pad = _ceil_to(self.npc, P)
        self.nblk = self.npc_pad // P
        self.n_pad = _ceil_to(n_nodes, P)

        src = np.asarray(src).astype(np.int64)
        dst = np.asarray(dst).astype(np.int64)
        self.src, self.dst = src, dst

        # ---- phase 1: bucket edges by (dst core, dst block); lo/hi by src
        core1 = dst // self.npc
        blk1 = (dst - core1 * self.npc) // P
        is_lo = src < lo_limit
        self.p1 = []           # [core][block] -> (lo_ids, hi_ids)
        tl, th = 1, 1
        for c in range(nc):
            in_c = core1 == c
            blocks = []
            for b in range(self.nblk):
                m = in_c & (blk1 == b)
                lo = np.nonzero(m & is_lo)[0]
                hi = np.nonzero(m & ~is_lo)[0]
                blocks.append((lo, hi))
                tl = max(tl, -(-len(lo) // P))
                th = max(th, -(-len(hi) // P))
            self.p1.append(blocks)
        self.tl, self.th = tl, th
        self.t_blk = _ceil_to(tl + th, 4)       # tiles per block, mult of 4
        self.t1 = self.nblk * self.t_blk
        self.e1 = self.t1 * P

        # ---- phase 2: bucket edges by (src core, src block)
        core2 = src // self.npc
        blk2 = (src - core2 * self.npc) // P
        t2b = 1
        self.p2 = []           # [core][block] -> ids
        for c in range(nc):
            in_c = core2 == c
            blocks = []
            for b in range(self.nblk):
                ids = np.nonzero(in_c & (blk2 == b))[0]
                blocks.append(ids)
                t2b = max(t2b, -(-len(ids) // P))
            self.p2.append(blocks)
        self.t2blk = t2b
        self.t2 = _ceil_to(self.nblk * self.t2blk, 32)   # mult of 32 (superchunks)
        self.n_sc = self.t2 // 32
        self.e2 = self.t2 * P

    # ---- per-core input arrays -------------------------------------------
    def core_inputs(self, c, x_bf, h, snorm_n):
        p = self
        f32 = np.float32

        # phase-1 slots
        slots = np.full(p.e1, -1, dtype=np.int64)
        for b, (lo, hi) in enumerate(p.p1[c]):
            base = b * p.t_blk * P
            slots[base: base + len(lo)] = lo
            slots[base + p.tl * P: base + p.tl * P + len(hi)] = hi
        pad = slots < 0
        e_ids = np.where(pad, 0, slots)

        h_t = np.ascontiguousarray(h[e_ids].T).astype(BF16)
        h_t[:, pad] = BF16(0.0)

        dst_rel = (self.dst[e_ids] - c * p.npc - (np.arange(p.e1) // (p.t_blk * P)) * P)
        dst_rel = np.where(pad, -1.0, dst_rel.astype(f32)).astype(f32)
        dstrel_col = dst_rel.reshape(p.t1, P).T.copy()                # [128, t1] f32
        dstrel_row = dst_rel.astype(BF16).reshape(1, p.e1)            # [1, e1]

        src1 = np.where(pad, 0, self.src[e_ids])
        in_hi = np.zeros(p.e1, dtype=bool)
        for b in range(p.nblk):
            s = b * p.t_blk * P + p.tl * P
            in_hi[s: b * p.t_blk * P + p.t_blk * P] = True
        idx = np.where(in_hi, np.maximum(src1 - p.lo_limit, 0), src1)
        idx = np.where(pad, 0, idx)

        # phase-2 slots
        slots2 = np.full(p.e2, -1, dtype=np.int64)
        for b, ids in enumerate(p.p2[c]):
            base = b * p.t2blk * P
            slots2[base: base + len(ids)] = ids
        pad2 = slots2 < 0
        e2_ids = np.where(pad2, 0, slots2)
        src_rel = self.src[e2_ids] - c * p.npc - \
            np.minimum(np.arange(p.e2) // (p.t2blk * P), p.nblk - 1) * P
        src_rel = np.where(pad2, -1.0, src_rel.astype(f32)).astype(f32)
        srcrel_row = src_rel.astype(BF16).reshape(1, p.e2)            # [1, e2]

        s = snorm_n.reshape(-1)[e2_ids].astype(np.float64)
        with np.errstate(divide="ignore", over="ignore"):
            c2x = P * LN_EPS / (s * s)          # 128 * eps / s^2  (inf ok)
        c2x = np.where(pad2, 1.0, c2x).astype(f32)
        c2c = c2x.reshape(p.t2, P).T.copy()             # [128, t2] columns

        return {
            "h_t": h_t,
            "dstrel_col": dstrel_col,
            "dstrel_row": dstrel_row,
            "idx_xb": _wrap16(idx),
            "srcrel_row": srcrel_row,
            "c2c": c2c,
            "x_tl": np.ascontiguousarray(
                x_bf[c * p.npc: (c + 1) * p.npc].T) if p.npc == p.npc_pad else
                np.ascontiguousarray(np.pad(
                    x_bf[c * p.npc: (c + 1) * p.npc],
                    ((0, p.npc_pad - p.npc), (0, 0))).T),
        }, slots2


# ----------------------------------------------------------------------------
# bass program
# ----------------------------------------------------------------------------


def build_program(p: Plan, use_gamma, use_beta):
    dt = mybir.dt
    nc = bacc.Bacc(None)
    nc.gpsimd.load_library(library_config.mlp)

    f32, bf16 = dt.float32, dt.bfloat16
    lo_rows = min(p.lo_limit, p.n_pad)
    REC = 132          # rhs_blk row: [relu(yc*gamma)(128) | 128*var | pad]

    x_nm = nc.declare_dram_parameter("x_nm", [p.n_pad, P], bf16, isOutput=False)
    x_tl = nc.declare_dram_parameter("x_tl", [P, p.npc_pad], bf16, isOutput=False)
    h_t = nc.declare_dram_parameter("h_t", [P, p.e1], bf16, isOutput=False)
    w1aT = nc.declare_dram_parameter("w1aT", [P, P], bf16, isOutput=False)
    w1bT = nc.declare_dram_parameter("w1bT", [P, P], bf16, isOutput=False)
    w1cT = nc.declare_dram_parameter("w1cT", [P, P], bf16, isOutput=False)
    w2T = nc.declare_dram_parameter("w2T", [P, P], bf16, isOutput=False)
    dstrel_col_in = nc.declare_dram_parameter("dstrel_col", [P, p.t1], f32, isOutput=False)
    dstrel_row_in = nc.declare_dram_parameter("dstrel_row", [1, p.e1], bf16, isOutput=False)
    srcrel_row_in = nc.declare_dram_parameter("srcrel_row", [1, p.e2], bf16, isOutput=False)
    idx_xb = nc.declare_dram_parameter("idx_xb", [P, p.e1 // 16], dt.int16, isOutput=False)
    c2c_in = nc.declare_dram_parameter("c2c", [P, p.t2], f32, isOutput=False)
    ones_in = nc.declare_dram_parameter("ones_row", [1, P], bf16, isOutput=False)
    iota_in = nc.declare_dram_parameter("iota_col", [P, 1], f32, isOutput=False)
    iota_row_in = nc.declare_dram_parameter("iota_row", [P, P], bf16, isOutput=False)
    gamma_b = beta_b = None
    if use_gamma:
        gamma_b = nc.declare_dram_parameter("gamma_b", [P, P], f32, isOutput=False)
    if use_beta:
        beta_b = nc.declare_dram_parameter("beta_b", [P, P], f32, isOutput=False)

    out = nc.declare_dram_parameter("out", [p.e2, P], bf16, isOutput=True)

    GMAX = 4

    def gather_tiles(out_tile, in_ap, idx_sb, slot0, n_tiles, tile_off):
        for g0 in range(0, n_tiles, GMAX):
            gn = min(GMAX, n_tiles - g0)
            e0 = slot0 + g0 * P
            nc.gpsimd.dma_gather(
                out_ap=out_tile[:, :, (tile_off + g0) * P:(tile_off + g0 + gn) * P],
                in_ap=in_ap,
                idxs_ap=idx_sb[:, e0 // 16: (e0 + gn * P) // 16],
                num_idxs=gn * P, num_idxs_reg=gn * P, elem_size=P,
                transpose=True)

    with tile.TileContext(nc) as tc:
        with tc.tile_pool(name="const", bufs=1) as cpool, \
             tc.tile_pool(name="hx", bufs=2) as hpool, \
             tc.tile_pool(name="gat", bufs=2) as gpool, \
             tc.tile_pool(name="row", bufs=2) as rpool, \
             tc.tile_pool(name="edge", bufs=3) as epool, \
             tc.tile_pool(name="blk", bufs=2) as bpool, \
             tc.tile_pool(name="oh2", bufs=9) as o2pool, \
             tc.tile_pool(name="p2s", bufs=2) as s2pool, \
             tc.tile_pool(name="outp", bufs=2) as opool, \
             tc.tile_pool(name="psA", bufs=2, space="PSUM") as psA, \
             tc.tile_pool(name="psB", bufs=2, space="PSUM") as psB, \
             tc.tile_pool(name="psC", bufs=2, space="PSUM") as psC:

            # ---- constants
            w1aT_sb = cpool.tile([P, P], bf16, tag="w1a")
            w1bT_sb = cpool.tile([P, P], bf16, tag="w1b")
            w1cT_sb = cpool.tile([P, P], bf16, tag="w1c")
            w2T_sb = cpool.tile([P, P], bf16, tag="w2")
            ones_sb = cpool.tile([1, P], bf16, tag="ones")
            iotac_sb = cpool.tile([P, 1], f32, tag="iotac")
            iotar_sb = cpool.tile([P, P], bf16, tag="iotar")
            dcol_sb = cpool.tile([P, p.t1], f32, tag="dcol")
            ixb_sb = cpool.tile([P, p.e1 // 16], dt.int16, tag="ixb")
            c2c_sb = cpool.tile([P, p.t2], f32, tag="c2c")
            nc.sync.dma_start(out=w1aT_sb[:], in_=w1aT[:])
            nc.sync.dma_start(out=w1bT_sb[:], in_=w1bT[:])
            nc.sync.dma_start(out=w1cT_sb[:], in_=w1cT[:])
            nc.sync.dma_start(out=w2T_sb[:], in_=w2T[:])
            nc.sync.dma_start(out=ones_sb[:], in_=ones_in[:])
            nc.sync.dma_start(out=iotac_sb[:], in_=iota_in[:])
            nc.sync.dma_start(out=iotar_sb[:], in_=iota_row_in[:])
            nc.sync.dma_start(out=dcol_sb[:], in_=dstrel_col_in[:])
            nc.sync.dma_start(out=ixb_sb[:], in_=idx_xb[:])
            nc.sync.dma_start(out=c2c_sb[:], in_=c2c_in[:])
            gamma_sb = beta_sb = None
            if use_gamma:
                gamma_sb = cpool.tile([P, P], f32, tag="gam")
                nc.sync.dma_start(out=gamma_sb[:], in_=gamma_b[:])
            if use_beta:
                beta_sb = cpool.tile([P, P], f32, tag="bet")
                nc.sync.dma_start(out=beta_sb[:], in_=beta_b[:])

            # per-block records, persistent
            rhs_blk = [cpool.tile([P, REC], bf16, tag=f"rec{b}",
                                  name=f"rec{b}")
                       for b in range(p.nblk)]

            scratch = cpool.tile([P, P], f32, tag="scr")
            n4 = p.t_blk // 4

            # ================= phase 1 =================
            for b in range(p.nblk):
                base_e = b * p.t_blk * P

                h_sb = hpool.tile([P, p.t_blk * P], bf16, tag="hblk")
                nc.sync.dma_start(out=h_sb[:],
                                  in_=h_t[:, base_e: base_e + p.t_blk * P])
                dr_sb = rpool.tile([1, p.t_blk * P], bf16, tag="drow")
                nc.sync.dma_start(out=dr_sb[:],
                                  in_=dstrel_row_in[:, base_e: base_e + p.t_blk * P])

                xg = gpool.tile([P, 1, p.t_blk * P], bf16, tag="xg")
                gather_tiles(xg, x_nm[:lo_rows, :], ixb_sb, base_e, p.tl, 0)
                if p.t_blk > p.tl:
                    gather_tiles(xg, x_nm[lo_rows:, :], ixb_sb,
                                 base_e + p.tl * P, p.t_blk - p.tl, p.tl)

                xt_b = hpool.tile([P, P], bf16, tag="xtb")
                nc.sync.dma_start(out=xt_b[:], in_=x_tl[:, b * P:(b + 1) * P])
                ps_xa = psC.tile([P, P], f32, tag="psxa")
                nc.tensor.matmul(out=ps_xa[:], lhsT=xt_b[:], rhs=w1aT_sb[:],
                                 start=True, stop=True)
                xa_sb = bpool.tile([P, P], bf16, tag="xasb")
                nc.scalar.copy(out=xa_sb[:], in_=ps_xa[:])

                ps_seg = psC.tile([P, P], f32, tag="seg")
                for g in range(n4):
                    t0 = g * 4
                    c0 = t0 * P
                    # broadcast dst_rel row -> [128, 512] psum (bf16)
                    bc = psB.tile([P, 4 * P], f32, tag="bc")
                    nc.tensor.matmul(out=bc[:], lhsT=ones_sb[:],
                                     rhs=dr_sb[:, c0:c0 + 4 * P],
                                     start=True, stop=True)
                    # ohT[node, e] = (node == dst_rel[e])
                    ohT = epool.tile([P, 4 * P], bf16, tag="ohT")
                    nc.vector.tensor_scalar(
                        out=ohT[:], in0=bc[:], scalar1=iotac_sb[:],
                        scalar2=None, op0=mybir.AluOpType.is_equal)
                    # oh[e, node] = (iota == dst_rel[e]) per tile
                    oh4 = epool.tile([P, 4, P], bf16, tag="oh4")
                    for tt in range(4):
                        nc.vector.tensor_scalar(
                            out=oh4[:, tt, :], in0=iotar_sb[:],
                            scalar1=dcol_sb[:, b * p.t_blk + t0 + tt:
                                            b * p.t_blk + t0 + tt + 1],
                            scalar2=None, op0=mybir.AluOpType.is_equal)
                    ps4 = psA.tile([P, 4, P], f32, tag="m4")
                    for tt in range(4):
                        e0 = base_e + c0 + tt * P
                        nc.tensor.matmul(out=ps4[:, tt, :],
                                         lhsT=h_sb[:, c0 + tt * P: c0 + (tt + 1) * P],
                                         rhs=w1cT_sb[:], start=True, stop=False)
                        nc.tensor.matmul(out=ps4[:, tt, :],
                                         lhsT=xg[:, 0, c0 + tt * P: c0 + (tt + 1) * P],
                                         rhs=w1bT_sb[:], start=False, stop=False)
                        nc.tensor.matmul(out=ps4[:, tt, :],
                                         lhsT=ohT[:, tt * P:(tt + 1) * P],
                                         rhs=xa_sb[:], start=False, stop=True)
                    me4 = epool.tile([P, 4, P], bf16, tag="me4")
                    nc.scalar.activation(out=me4[:], in_=ps4[:],
                                         func=mybir.ActivationFunctionType.Relu)
                    for tt in range(4):
                        nc.tensor.matmul(out=ps_seg[:], lhsT=me4[:, tt, :],
                                         rhs=oh4[:, tt, :],
                                         start=(g == 0 and tt == 0),
                                         stop=(g == n4 - 1 and tt == 3))

                # ---- phase 1.5
                mnT = bpool.tile([P, P], bf16, tag="mnT")
                nc.vector.tensor_copy(out=mnT[:], in_=ps_seg[:])
                ps_y = psC.tile([P, P], f32, tag="psxa")
                nc.tensor.matmul(out=ps_y[:], lhsT=mnT[:], rhs=w2T_sb[:],
                                 start=True, stop=True)
                summ = bpool.tile([P, 1], f32, tag="summ")
                nc.scalar.activation(out=scratch[:], in_=ps_y[:],
                                     func=mybir.ActivationFunctionType.Copy,
                                     accum_out=summ[:])
                sumsq = bpool.tile([P, 1], f32, tag="sumsq")
                nc.scalar.activation(out=scratch[:], in_=ps_y[:],
                                     func=mybir.ActivationFunctionType.Square,
                                     accum_out=sumsq[:])
                negmu = bpool.tile([P, 1], f32, tag="negmu")
                nc.vector.tensor_scalar_mul(negmu[:], summ[:], -1.0 / P)
                musq = bpool.tile([P, 1], f32, tag="musq")
                nc.vector.tensor_tensor(out=musq[:], in0=summ[:], in1=summ[:],
                                        op=mybir.AluOpType.mult)
                # 128*var = sumsq - musq/128
                nc.vector.scalar_tensor_tensor(
                    out=rhs_blk[b][:, P:P + 1], in0=musq[:], scalar=-1.0 / P,
                    in1=sumsq[:], op0=mybir.AluOpType.mult,
                    op1=mybir.AluOpType.add)
                if not use_beta:
                    if use_gamma:
                        yc = bpool.tile([P, P], f32, tag="ycg")
                        nc.scalar.activation(
                            out=yc[:], in_=ps_y[:],
                            func=mybir.ActivationFunctionType.Identity,
                            bias=negmu[:])
                        nc.vector.tensor_tensor(
                            out=scratch[:], in0=yc[:], in1=gamma_sb[:],
                            op=mybir.AluOpType.mult)
                        nc.scalar.activation(
                            out=rhs_blk[b][:, 0:P], in_=scratch[:],
                            func=mybir.ActivationFunctionType.Relu)
                    else:
                        nc.scalar.activation(
                            out=rhs_blk[b][:, 0:P], in_=ps_y[:],
                            func=mybir.ActivationFunctionType.Relu,
                            bias=negmu[:])
                else:
                    # general path: store yc*gamma (no relu)
                    yc = bpool.tile([P, P], f32, tag="ycg")
                    nc.scalar.activation(
                        out=yc[:], in_=ps_y[:],
                        func=mybir.ActivationFunctionType.Identity,
                        bias=negmu[:])
                    if use_gamma:
                        nc.vector.tensor_tensor(
                            out=rhs_blk[b][:, 0:P], in0=yc[:], in1=gamma_sb[:],
                            op=mybir.AluOpType.mult)
                    else:
                        nc.vector.tensor_copy(out=rhs_blk[b][:, 0:P], in_=yc[:])

            # ================= phase 2 =================
            for s in range(p.n_sc):
                e0s = s * 32 * P
                sr_sb = rpool.tile([1, 32 * P], bf16, tag="srow")
                nc.sync.dma_start(out=sr_sb[:],
                                  in_=srcrel_row_in[:, e0s: e0s + 32 * P])

                psVc = psC.tile([P, 32], f32, tag="psxa")
                ohTs = []
                for g in range(8):
                    c0 = g * 4 * P
                    bc2 = psB.tile([P, 4 * P], f32, tag="bc")
                    nc.tensor.matmul(out=bc2[:], lhsT=ones_sb[:],
                                     rhs=sr_sb[:, c0:c0 + 4 * P],
                                     start=True, stop=True)
                    ohT2 = o2pool.tile([P, 4 * P], bf16, tag="ohT2")
                    nc.vector.tensor_scalar(
                        out=ohT2[:], in0=bc2[:], scalar1=iotac_sb[:],
                        scalar2=None, op0=mybir.AluOpType.is_equal)
                    ohTs.append(ohT2)
                    for tt in range(4):
                        t = s * 32 + g * 4 + tt
                        b = min(t // p.t2blk, p.nblk - 1)
                        nc.tensor.matmul(
                            out=psVc[:, g * 4 + tt: g * 4 + tt + 1],
                            lhsT=ohT2[:, tt * P:(tt + 1) * P],
                            rhs=rhs_blk[b][:, P:P + 1],
                            start=True, stop=True)
                # a = 1/sqrt((varx + c2x)/128), per edge, column layout
                vc = s2pool.tile([P, 32], f32, tag="vc")
                nc.vector.tensor_tensor(out=vc[:], in0=psVc[:],
                                        in1=c2c_sb[:, s * 32:(s + 1) * 32],
                                        op=mybir.AluOpType.add)
                rt = s2pool.tile([P, 32], f32, tag="rt")
                nc.scalar.activation(out=rt[:], in_=vc[:],
                                     func=mybir.ActivationFunctionType.Sqrt,
                                     scale=1.0 / P)
                a_sb = s2pool.tile([P, 32], f32, tag="a_sb")
                nc.vector.reciprocal(out=a_sb[:], in_=rt[:])

                out_sb = opool.tile([P, 32, P], bf16, tag="osb")
                for g in range(8):
                    ohT2 = ohTs[g]
                    sel4 = psA.tile([P, 4, P], f32, tag="m4")
                    for tt in range(4):
                        t = s * 32 + g * 4 + tt
                        b = min(t // p.t2blk, p.nblk - 1)
                        nc.tensor.matmul(out=sel4[:, tt, :],
                                         lhsT=ohT2[:, tt * P:(tt + 1) * P],
                                         rhs=rhs_blk[b][:, 0:P],
                                         start=True, stop=True)
                    for tt in range(4):
                        tsc = g * 4 + tt
                        if not use_beta:
                            nc.scalar.activation(
                                out=out_sb[:, tsc, :], in_=sel4[:, tt, :],
                                func=mybir.ActivationFunctionType.Copy,
                                scale=a_sb[:, tsc:tsc + 1])
                        else:
                            tmp = s2pool.tile([P, P], f32, tag="tmpb")
                            nc.vector.tensor_scalar(
                                out=tmp[:], in0=sel4[:, tt, :],
                                scalar1=a_sb[:, tsc:tsc + 1], scalar2=None,
                                op0=mybir.AluOpType.mult)
                            nc.vector.tensor_tensor(
                                out=tmp[:], in0=tmp[:], in1=beta_sb[:],
                                op=mybir.AluOpType.add)
                            nc.scalar.activation(
                                out=out_sb[:, tsc, :], in_=tmp[:],
                                func=mybir.ActivationFunctionType.Relu)

                out_view = out[e0s: e0s + 32 * P, :].rearrange(
                    "(t p) d -> p t d", p=P)
                nc.sync.dma_start(out=out_view, in_=out_sb[:])

    nc.finalize()
    return nc


# ----------------------------------------------------------------------------
# driver
# ----------------------------------------------------------------------------


def _prep_inputs(p: Plan, x, h, snorm_n, W1, W2, ln_gamma, ln_beta):
    D = P
    use_gamma = not np.allclose(ln_gamma, 1.0)
    use_beta = not np.allclose(ln_beta, 0.0)

    x_bf = np.zeros((p.n_pad, D), dtype=BF16)
    x_bf[:p.n_nodes] = np.asarray(x).astype(BF16)

    common = {
        "x_nm": x_bf,
        "w1aT": np.ascontiguousarray(W1[:, :D].T).astype(BF16),
        "w1bT": np.ascontiguousarray(W1[:, D:2 * D].T).astype(BF16),
        "w1cT": np.ascontiguousarray(W1[:, 2 * D:3 * D].T).astype(BF16),
        "w2T": np.ascontiguousarray(W2.T).astype(BF16),
        "ones_row": np.ones((1, P), dtype=BF16),
        "iota_col": np.arange(P, dtype=np.float32).reshape(P, 1),
        "iota_row": np.tile(np.arange(P, dtype=np.float32), (P, 1)).astype(BF16),
    }
    if use_gamma:
        common["gamma_b"] = np.tile(np.asarray(ln_gamma, np.float32), (P, 1))
    if use_beta:
        common["beta_b"] = np.tile(np.asarray(ln_beta, np.float32), (P, 1))

    in_maps, slots2_all = [], []
    for c in range(p.nc):
        m, slots2 = p.core_inputs(c, x_bf, h, snorm_n)
        m.update(common)
        in_maps.append(m)
        slots2_all.append(slots2)
    return in_maps, slots2_all, use_gamma, use_beta


def run(x, h, snorm_n, W1, W2, ln_gamma, ln_beta, src, dst, n_cores=8,
        trace=False):
    n_nodes, n_edges = x.shape[0], h.shape[0]
    p = Plan(n_nodes, n_edges, src, dst, nc=n_cores)
    in_maps, slots2_all, use_gamma, use_beta = _prep_inputs(
        p, x, h, snorm_n, W1, W2, ln_gamma, ln_beta)
    nc = build_program(p, use_gamma, use_beta)
    res = run_bass_kernel_spmd(nc, in_maps, core_ids=list(range(n_cores)),
                               trace=trace)
    out = np.empty((n_edges, P), dtype=np.float32)
    for c in range(n_cores):
        o = res.results[c]["out"]
        s = slots2_all[c]
        real = s >= 0
        out[s[real]] = o[real].astype(np.float32)
    return out, res


def kernel(x, h, snorm_n, snorm_e, W1, W2, ln_gamma, ln_beta, src, dst):
    out, _ = run(np.asarray(x), np.asarray(h), np.asarray(snorm_n),
                 np.asarray(W1), np.asarray(W2), np.asarray(ln_gamma),
                 np.asarray(ln_beta), np.asarray(src), np.asarray(dst))
    return out


# revision 22
# speedup vs baseline: 1.8653x; 1.0172x over previous
"""MPNN layer on 8 Trainium2 NeuronCores (Bass/Tile) - v2, collective-free.

Math (reference):
    m_edge = relu(x[dst] @ W1a^T + x[src] @ W1b^T + h @ W1c^T)        [E, D]
    m_node = segment_sum(m_edge, dst, N)                               [N, D]
    y      = m_node @ W2^T                                             [N, D]
    out_e  = relu(LN(snorm_n_e * y[src_e]))                            [E, D]

LN decomposition (exact, s>0):
    LN(s*v)*gamma+beta = (v - mu_v)*gamma * a_e + beta,
    a_e = s*rsqrt(s^2*var_v + eps) = rsqrt(var_v + eps/s^2)
and for beta==0:  relu(yc*gamma * a_e) = a_e * relu(yc*gamma)  (a_e > 0),
so the per-node record stores relu'd centered y and var; the per-edge part is
a scalar multiply.

Sharding (no collectives, each core independent):
  phase 1: edges bucketed by dst-owner core + 128-node dst block. Segment-sum
    per block via one-hot matmuls. x[src] fetched by transpose dma_gather of
    the node-major bf16 x table (global); x[dst] selected on PE via one-hot;
    h term via per-tile matmul.
  phase 1.5: per block: y = m@W2, mean/var/relu-center; records stay in SBUF.
  phase 2: edges bucketed by src-owner core + src block (every core owns the
    records it needs). One-hot select of record + var on PE; a = rsqrt(...)
    computed on batched psum row-vectors; output written bf16 in bucketed
    order; host inverts the permutation.
"""

import numpy as np
import ml_dtypes

from concourse import bacc, tile, mybir
from concourse import library_config
from concourse.bass_utils import run_bass_kernel_spmd

P = 128
LN_EPS = 1e-5
BF16 = ml_dtypes.bfloat16

# ----------------------------------------------------------------------------
# host-side planning
# ----------------------------------------------------------------------------


def _ceil_to(x, m):
    return -(-x // m) * m


def _wrap16(idx, dtype=np.int16):
    """[n] -> [128, n//16] int16: idx i at partition i%16, col i//16, replicated
    over the 8 groups of 16 partitions."""
    n = idx.shape[0]
    assert n % 16 == 0
    w = idx.reshape(n // 16, 16).T.astype(dtype)
    return np.tile(w, (8, 1))


class Plan:
    def __init__(self, n_nodes, n_edges, src, dst, nc=8, lo_limit=32768):
        self.nc = nc
        self.n_nodes = n_nodes
        self.n_edges = n_edges
        self.lo_limit = lo_limit
        self.npc = n_nodes // nc
        assert self.npc * nc == n_nodes
        self.npc_pad = _ceil_to(self.npc, P)
        self.nblk = self.npc_pad // P
        self.n_pad = _ceil_to(n_nodes, P)

        src = np.asarray(src).astype(np.int64)
        dst = np.asarray(dst).astype(np.int64)
        self.src, self.dst = src, dst

        # ---- phase 1: bucket edges by (dst core, dst block); lo/hi by src
        core1 = dst // self.npc
        blk1 = (dst - core1 * self.npc) // P
        is_lo = src < lo_limit
        self.p1 = []           # [core][block] -> (lo_ids, hi_ids)
        tl, th = 1, 1
        for c in range(nc):
            in_c = core1 == c
            blocks = []
            for b in range(self.nblk):
                m = in_c & (blk1 == b)
                lo = np.nonzero(m & is_lo)[0]
                hi = np.nonzero(m & ~is_lo)[0]
                blocks.append((lo, hi))
                tl = max(tl, -(-len(lo) // P))
                th = max(th, -(-len(hi) // P))
            self.p1.append(blocks)
        self.tl, self.th = tl, th
        self.t_blk = _ceil_to(tl + th, 4)       # tiles per block, mult of 4
        self.t1 = self.nblk * self.t_blk
        self.e1 = self.t1 * P

        # ---- phase 2: bucket edges by (src core, src block)
        core2 = src // self.npc
        blk2 = (src - core2 * self.npc) // P
        t2b = 1
        self.p2 = []           # [core][block] -> ids
        for c in range(nc):
            in_c = core2 == c
            blocks = []
            for b in range(self.nblk):
                ids = np.nonzero(in_c & (blk2 == b))[0]
                blocks.append(ids)
                t2b = max(t2b, -(-len(ids) // P))
            self.p2.append(blocks)
        self.t2blk = t2b
        self.t2 = _ceil_to(self.nblk * self.t2blk, 32)   # mult of 32 (superchunks)
        self.n_sc = self.t2 // 32
        self.e2 = self.t2 * P

    # ---- per-core input arrays -------------------------------------------
    def core_inputs(self, c, x_bf, h, snorm_n):
        p = self
        f32 = np.float32

        # phase-1 slots
        slots = np.full(p.e1, -1, dtype=np.int64)
        for b, (lo, hi) in enumerate(p.p1[c]):
            base = b * p.t_blk * P
            slots[base: base + len(lo)] = lo
            slots[base + p.tl * P: base + p.tl * P + len(hi)] = hi
        pad = slots < 0
        e_ids = np.where(pad, 0, slots)

        h_t = np.ascontiguousarray(h[e_ids].T).astype(BF16)
        h_t[:, pad] = BF16(0.0)

        dst_rel = (self.dst[e_ids] - c * p.npc - (np.arange(p.e1) // (p.t_blk * P)) * P)
        dst_rel = np.where(pad, -1.0, dst_rel.astype(f32)).astype(f32)
        dstrel_col = dst_rel.reshape(p.t1, P).T.copy()                # [128, t1] f32
        dstrel_row = dst_rel.astype(BF16).reshape(1, p.e1)            # [1, e1]

        src1 = np.where(pad, 0, self.src[e_ids])
        in_hi = np.zeros(p.e1, dtype=bool)
        for b in range(p.nblk):
            s = b * p.t_blk * P + p.tl * P
            in_hi[s: b * p.t_blk * P + p.t_blk * P] = True
        idx = np.where(in_hi, np.maximum(src1 - p.lo_limit, 0), src1)
        idx = np.where(pad, 0, idx)

        # phase-2 slots
        slots2 = np.full(p.e2, -1, dtype=np.int64)
        for b, ids in enumerate(p.p2[c]):
            base = b * p.t2blk * P
            slots2[base: base + len(ids)] = ids
        pad2 = slots2 < 0
        e2_ids = np.where(pad2, 0, slots2)
        src_rel = self.src[e2_ids] - c * p.npc - \
            np.minimum(np.arange(p.e2) // (p.t2blk * P), p.nblk - 1) * P
        src_rel = np.where(pad2, -1.0, src_rel.astype(f32)).astype(f32)
        srcrel_row = src_rel.astype(BF16).reshape(1, p.e2)            # [1, e2]

        s = snorm_n.reshape(-1)[e2_ids].astype(np.float64)
        with np.errstate(divide="ignore", over="ignore"):
            c2x = P * LN_EPS / (s * s)          # 128 * eps / s^2  (inf ok)
        c2x = np.where(pad2, 1.0, c2x).astype(f32)
        c2c = c2x.reshape(p.t2, P).T.copy()             # [128, t2] columns

        return {
            "h_t": h_t,
            "dstrel_col": dstrel_col,
            "dstrel_row": dstrel_row,
            "idx_xb": _wrap16(idx),
            "srcrel_row": srcrel_row,
            "c2c": c2c,
            "x_tl": np.ascontiguousarray(
                x_bf[c * p.npc: (c + 1) * p.npc].T) if p.npc == p.npc_pad else
                np.ascontiguousarray(np.pad(
                    x_bf[c * p.npc: (c + 1) * p.npc],
                    ((0, p.npc_pad - p.npc), (0, 0))).T),
        }, slots2


# ----------------------------------------------------------------------------
# bass program
# ----------------------------------------------------------------------------


def build_program(p: Plan, use_gamma, use_beta):
    dt = mybir.dt
    nc = bacc.Bacc(None)
    nc.gpsimd.load_library(library_config.mlp)

    f32, bf16 = dt.float32, dt.bfloat16
    lo_rows = min(p.lo_limit, p.n_pad)
    REC = 132          # rhs_blk row: [relu(yc*gamma)(128) | 128*var | pad]

    x_t = nc.declare_dram_parameter("x_t", [P, p.n_pad], bf16, isOutput=False)
    x_tl = nc.declare_dram_parameter("x_tl", [P, p.npc_pad], bf16, isOutput=False)
    ident_in = nc.declare_dram_parameter("ident", [P, P], bf16, isOutput=False)
    xb_dram = nc.dram_tensor("xb_dram", [p.n_pad, P], bf16)
    h_t = nc.declare_dram_parameter("h_t", [P, p.e1], bf16, isOutput=False)
    w1aT = nc.declare_dram_parameter("w1aT", [P, P], bf16, isOutput=False)
    w1bT = nc.declare_dram_parameter("w1bT", [P, P], bf16, isOutput=False)
    w1cT = nc.declare_dram_parameter("w1cT", [P, P], bf16, isOutput=False)
    w2T = nc.declare_dram_parameter("w2T", [P, P], bf16, isOutput=False)
    dstrel_col_in = nc.declare_dram_parameter("dstrel_col", [P, p.t1], f32, isOutput=False)
    dstrel_row_in = nc.declare_dram_parameter("dstrel_row", [1, p.e1], bf16, isOutput=False)
    srcrel_row_in = nc.declare_dram_parameter("srcrel_row", [1, p.e2], bf16, isOutput=False)
    idx_xb = nc.declare_dram_parameter("idx_xb", [P, p.e1 // 16], dt.int16, isOutput=False)
    c2c_in = nc.declare_dram_parameter("c2c", [P, p.t2], f32, isOutput=False)
    ones_in = nc.declare_dram_parameter("ones_row", [1, P], bf16, isOutput=False)
    iota_in = nc.declare_dram_parameter("iota_col", [P, 1], f32, isOutput=False)
    iota_row_in = nc.declare_dram_parameter("iota_row", [P, P], bf16, isOutput=False)
    gamma_b = beta_b = None
    if use_gamma:
        gamma_b = nc.declare_dram_parameter("gamma_b", [P, P], f32, isOutput=False)
    if use_beta:
        beta_b = nc.declare_dram_parameter("beta_b", [P, P], f32, isOutput=False)

    out = nc.declare_dram_parameter("out", [p.e2, P], bf16, isOutput=True)

    GMAX = 8

    def gather_tiles(out_tile, in_ap, idx_sb, slot0, n_tiles, tile_off):
        for g0 in range(0, n_tiles, GMAX):
            gn = min(GMAX, n_tiles - g0)
            e0 = slot0 + g0 * P
            nc.gpsimd.dma_gather(
                out_ap=out_tile[:, tile_off + g0: tile_off + g0 + gn, :],
                in_ap=in_ap,
                idxs_ap=idx_sb[:, e0 // 16: (e0 + gn * P) // 16],
                num_idxs=gn * P, num_idxs_reg=gn * P, elem_size=P)

    with tile.TileContext(nc) as tc:
        with tc.tile_pool(name="const", bufs=1) as cpool, \
             tc.tile_pool(name="hx", bufs=2) as hpool, \
             tc.tile_pool(name="gat", bufs=2) as gpool, \
             tc.tile_pool(name="row", bufs=2) as rpool, \
             tc.tile_pool(name="edge", bufs=3) as epool, \
             tc.tile_pool(name="blk", bufs=2) as bpool, \
             tc.tile_pool(name="oh2", bufs=9) as o2pool, \
             tc.tile_pool(name="p2s", bufs=2) as s2pool, \
             tc.tile_pool(name="outp", bufs=2) as opool, \
             tc.tile_pool(name="psA", bufs=2, space="PSUM") as psA, \
             tc.tile_pool(name="psB", bufs=2, space="PSUM") as psB, \
             tc.tile_pool(name="psC", bufs=2, space="PSUM") as psC:

            # ---- constants
            w1aT_sb = cpool.tile([P, P], bf16, tag="w1a")
            w1bT_sb = cpool.tile([P, P], bf16, tag="w1b")
            w1cT_sb = cpool.tile([P, P], bf16, tag="w1c")
            w2T_sb = cpool.tile([P, P], bf16, tag="w2")
            ones_sb = cpool.tile([1, P], bf16, tag="ones")
            iotac_sb = cpool.tile([P, 1], f32, tag="iotac")
            iotar_sb = cpool.tile([P, P], bf16, tag="iotar")
            dcol_sb = cpool.tile([P, p.t1], f32, tag="dcol")
            ixb_sb = cpool.tile([P, p.e1 // 16], dt.int16, tag="ixb")
            c2c_sb = cpool.tile([P, p.t2], f32, tag="c2c")
            nc.sync.dma_start(out=w1aT_sb[:], in_=w1aT[:])
            nc.sync.dma_start(out=w1bT_sb[:], in_=w1bT[:])
            nc.sync.dma_start(out=w1cT_sb[:], in_=w1cT[:])
            nc.sync.dma_start(out=w2T_sb[:], in_=w2T[:])
            nc.sync.dma_start(out=ones_sb[:], in_=ones_in[:])
            nc.sync.dma_start(out=iotac_sb[:], in_=iota_in[:])
            nc.sync.dma_start(out=iotar_sb[:], in_=iota_row_in[:])
            nc.sync.dma_start(out=dcol_sb[:], in_=dstrel_col_in[:])
            nc.sync.dma_start(out=ixb_sb[:], in_=idx_xb[:])
            nc.sync.dma_start(out=c2c_sb[:], in_=c2c_in[:])
            ident_sb = cpool.tile([P, P], bf16, tag="ident")
            nc.sync.dma_start(out=ident_sb[:], in_=ident_in[:])
            gamma_sb = beta_sb = None
            if use_gamma:
                gamma_sb = cpool.tile([P, P], f32, tag="gam")
                nc.sync.dma_start(out=gamma_sb[:], in_=gamma_b[:])
            if use_beta:
                beta_sb = cpool.tile([P, P], f32, tag="bet")
                nc.sync.dma_start(out=beta_sb[:], in_=beta_b[:])

            # per-block records, persistent
            rhs_blk = [cpool.tile([P, REC], bf16, tag=f"rec{b}",
                                  name=f"rec{b}")
                       for b in range(p.nblk)]

            scratch = cpool.tile([P, P], f32, tag="scr")
            n4 = p.t_blk // 4

            # ================= phase 0: xb table = x @ W1b^T ===============
            n_xt = p.n_pad // P
            W16 = 16
            j = 0
            while j < n_xt:
                w = min(W16, n_xt - j)
                xt_sb = gpool.tile([P, W16 * P], bf16, tag="xtsb")
                nc.sync.dma_start(out=xt_sb[:, 0:w * P],
                                  in_=x_t[:, j * P:(j + w) * P])
                xbt = hpool.tile([P, W16, P], bf16, tag="xbt")
                for q in range(0, w, 4):
                    qw = min(4, w - q)
                    ps_t = psA.tile([P, 4, P], f32, tag="m4")
                    for tt in range(qw):
                        nc.tensor.matmul(out=ps_t[:, tt, :],
                                         lhsT=xt_sb[:, (q + tt) * P:(q + tt + 1) * P],
                                         rhs=w1bT_sb[:], start=True, stop=True)
                    nc.scalar.activation(
                        out=xbt[:, q:q + qw, :].rearrange("p a b -> p (a b)"),
                        in_=ps_t[:, 0:qw, :].rearrange("p a b -> p (a b)"),
                        func=mybir.ActivationFunctionType.Copy)
                xb_view = xb_dram[j * P:(j + w) * P, :].rearrange(
                    "(t p) d -> p t d", p=P)
                nc.sync.dma_start(out=xb_view, in_=xbt[:, 0:w, :])
                j += w

            # ================= phase 1 =================
            for b in range(p.nblk):
                base_e = b * p.t_blk * P

                h_sb = hpool.tile([P, p.t_blk * P], bf16, tag="hblk")
                nc.sync.dma_start(out=h_sb[:],
                                  in_=h_t[:, base_e: base_e + p.t_blk * P])
                dr_sb = rpool.tile([1, p.t_blk * P], bf16, tag="drow")
                nc.sync.dma_start(out=dr_sb[:],
                                  in_=dstrel_row_in[:, base_e: base_e + p.t_blk * P])

                xg = gpool.tile([P, p.t_blk, P], bf16, tag="xg")
                gather_tiles(xg, xb_dram[:lo_rows, :], ixb_sb, base_e, p.tl, 0)
                if p.t_blk > p.tl:
                    gather_tiles(xg, xb_dram[lo_rows:, :], ixb_sb,
                                 base_e + p.tl * P, p.t_blk - p.tl, p.tl)

                xt_b = hpool.tile([P, P], bf16, tag="xtb")
                nc.sync.dma_start(out=xt_b[:], in_=x_tl[:, b * P:(b + 1) * P])
                ps_xa = psC.tile([P, P], f32, tag="psxa")
                nc.tensor.matmul(out=ps_xa[:], lhsT=xt_b[:], rhs=w1aT_sb[:],
                                 start=True, stop=True)
                xa_sb = bpool.tile([P, P], bf16, tag="xasb")
                nc.scalar.copy(out=xa_sb[:], in_=ps_xa[:])

                ps_seg = psC.tile([P, P], f32, tag="seg")
                for g in range(n4):
                    t0 = g * 4
                    c0 = t0 * P
                    # broadcast dst_rel row -> [128, 512] psum (bf16)
                    bc = psB.tile([P, 4 * P], f32, tag="bc")
                    nc.tensor.matmul(out=bc[:], lhsT=ones_sb[:],
                                     rhs=dr_sb[:, c0:c0 + 4 * P],
                                     start=True, stop=True)
                    # ohT[node, e] = (node == dst_rel[e])
                    ohT = epool.tile([P, 4 * P], bf16, tag="ohT")
                    nc.vector.tensor_scalar(
                        out=ohT[:], in0=bc[:], scalar1=iotac_sb[:],
                        scalar2=None, op0=mybir.AluOpType.is_equal)
                    # oh[e, node] = (iota == dst_rel[e]) per tile
                    oh4 = epool.tile([P, 4, P], bf16, tag="oh4")
                    for tt in range(4):
                        nc.vector.tensor_scalar(
                            out=oh4[:, tt, :], in0=iotar_sb[:],
                            scalar1=dcol_sb[:, b * p.t_blk + t0 + tt:
                                            b * p.t_blk + t0 + tt + 1],
                            scalar2=None, op0=mybir.AluOpType.is_equal)
                    ps4 = psA.tile([P, 4, P], f32, tag="m4")
                    for tt in range(4):
                        e0 = base_e + c0 + tt * P
                        nc.tensor.matmul(out=ps4[:, tt, :],
                                         lhsT=h_sb[:, c0 + tt * P: c0 + (tt + 1) * P],
                                         rhs=w1cT_sb[:], start=True, stop=False)
                        nc.tensor.matmul(out=ps4[:, tt, :],
                                         lhsT=ident_sb[:],
                                         rhs=xg[:, t0 + tt, :],
                                         start=False, stop=False)
                        nc.tensor.matmul(out=ps4[:, tt, :],
                                         lhsT=ohT[:, tt * P:(tt + 1) * P],
                                         rhs=xa_sb[:], start=False, stop=True)
                    me4 = epool.tile([P, 4, P], bf16, tag="me4")
                    nc.scalar.activation(out=me4[:], in_=ps4[:],
                                         func=mybir.ActivationFunctionType.Relu)
                    for tt in range(4):
                        nc.tensor.matmul(out=ps_seg[:], lhsT=me4[:, tt, :],
                                         rhs=oh4[:, tt, :],
                                         start=(g == 0 and tt == 0),
                                         stop=(g == n4 - 1 and tt == 3))

                # ---- phase 1.5
                mnT = bpool.tile([P, P], bf16, tag="mnT")
                nc.vector.tensor_copy(out=mnT[:], in_=ps_seg[:])
                ps_y = psC.tile([P, P], f32, tag="psxa")
                nc.tensor.matmul(out=ps_y[:], lhsT=mnT[:], rhs=w2T_sb[:],
                                 start=True, stop=True)
                summ = bpool.tile([P, 1], f32, tag="summ")
                nc.scalar.activation(out=scratch[:], in_=ps_y[:],
                                     func=mybir.ActivationFunctionType.Copy,
                                     accum_out=summ[:])
                sumsq = bpool.tile([P, 1], f32, tag="sumsq")
                nc.scalar.activation(out=scratch[:], in_=ps_y[:],
                                     func=mybir.ActivationFunctionType.Square,
                                     accum_out=sumsq[:])
                negmu = bpool.tile([P, 1], f32, tag="negmu")
                nc.vector.tensor_scalar_mul(negmu[:], summ[:], -1.0 / P)
                musq = bpool.tile([P, 1], f32, tag="musq")
                nc.vector.tensor_tensor(out=musq[:], in0=summ[:], in1=summ[:],
                                        op=mybir.AluOpType.mult)
                # 128*var = sumsq - musq/128
                nc.vector.scalar_tensor_tensor(
                    out=rhs_blk[b][:, P:P + 1], in0=musq[:], scalar=-1.0 / P,
                    in1=sumsq[:], op0=mybir.AluOpType.mult,
                    op1=mybir.AluOpType.add)
                if not use_beta:
                    if use_gamma:
                        yc = bpool.tile([P, P], f32, tag="ycg")
                        nc.scalar.activation(
                            out=yc[:], in_=ps_y[:],
                            func=mybir.ActivationFunctionType.Identity,
                            bias=negmu[:])
                        nc.vector.tensor_tensor(
                            out=scratch[:], in0=yc[:], in1=gamma_sb[:],
                            op=mybir.AluOpType.mult)
                        nc.scalar.activation(
                            out=rhs_blk[b][:, 0:P], in_=scratch[:],
                            func=mybir.ActivationFunctionType.Relu)
                    else:
                        nc.scalar.activation(
                            out=rhs_blk[b][:, 0:P], in_=ps_y[:],
                            func=mybir.ActivationFunctionType.Relu,
                            bias=negmu[:])
                else:
                    # general path: store yc*gamma (no relu)
                    yc = bpool.tile([P, P], f32, tag="ycg")
                    nc.scalar.activation(
                        out=yc[:], in_=ps_y[:],
                        func=mybir.ActivationFunctionType.Identity,
                        bias=negmu[:])
                    if use_gamma:
                        nc.vector.tensor_tensor(
                            out=rhs_blk[b][:, 0:P], in0=yc[:], in1=gamma_sb[:],
                            op=mybir.AluOpType.mult)
                    else:
                        nc.vector.tensor_copy(out=rhs_blk[b][:, 0:P], in_=yc[:])

            # ================= phase 2 =================
            for s in range(p.n_sc):
                e0s = s * 32 * P
                sr_sb = rpool.tile([1, 32 * P], bf16, tag="srow")
                nc.sync.dma_start(out=sr_sb[:],
                                  in_=srcrel_row_in[:, e0s: e0s + 32 * P])

                psVc = psC.tile([P, 32], f32, tag="psxa")
                ohTs = []
                for g in range(8):
                    c0 = g * 4 * P
                    bc2 = psB.tile([P, 4 * P], f32, tag="bc")
                    nc.tensor.matmul(out=bc2[:], lhsT=ones_sb[:],
                                     rhs=sr_sb[:, c0:c0 + 4 * P],
                                     start=True, stop=True)
                    ohT2 = o2pool.tile([P, 4 * P], bf16, tag="ohT2")
                    nc.vector.tensor_scalar(
                        out=ohT2[:], in0=bc2[:], scalar1=iotac_sb[:],
                        scalar2=None, op0=mybir.AluOpType.is_equal)
                    ohTs.append(ohT2)
                    for tt in range(4):
                        t = s * 32 + g * 4 + tt
                        b = min(t // p.t2blk, p.nblk - 1)
                        nc.tensor.matmul(
                            out=psVc[:, g * 4 + tt: g * 4 + tt + 1],
                            lhsT=ohT2[:, tt * P:(tt + 1) * P],
                            rhs=rhs_blk[b][:, P:P + 1],
                            start=True, stop=True)
                # a = 1/sqrt((varx + c2x)/128), per edge, column layout
                vc = s2pool.tile([P, 32], f32, tag="vc")
                nc.vector.tensor_tensor(out=vc[:], in0=psVc[:],
                                        in1=c2c_sb[:, s * 32:(s + 1) * 32],
                                        op=mybir.AluOpType.add)
                rt = s2pool.tile([P, 32], f32, tag="rt")
                nc.scalar.activation(out=rt[:], in_=vc[:],
                                     func=mybir.ActivationFunctionType.Sqrt,
                                     scale=1.0 / P)
                a_sb = s2pool.tile([P, 32], f32, tag="a_sb")
                nc.vector.reciprocal(out=a_sb[:], in_=rt[:])

                out_sb = opool.tile([P, 32, P], bf16, tag="osb")
                for g in range(8):
                    ohT2 = ohTs[g]
                    sel4 = psA.tile([P, 4, P], f32, tag="m4")
                    for tt in range(4):
                        t = s * 32 + g * 4 + tt
                        b = min(t // p.t2blk, p.nblk - 1)
                        nc.tensor.matmul(out=sel4[:, tt, :],
                                         lhsT=ohT2[:, tt * P:(tt + 1) * P],
                                         rhs=rhs_blk[b][:, 0:P],
                                         start=True, stop=True)
                    for tt in range(4):
                        tsc = g * 4 + tt
                        if not use_beta:
                            if tt % 2 == 0:
                                nc.scalar.activation(
                                    out=out_sb[:, tsc, :], in_=sel4[:, tt, :],
                                    func=mybir.ActivationFunctionType.Copy,
                                    scale=a_sb[:, tsc:tsc + 1])
                            else:
                                nc.vector.tensor_scalar(
                                    out=out_sb[:, tsc, :], in0=sel4[:, tt, :],
                                    scalar1=a_sb[:, tsc:tsc + 1], scalar2=None,
                                    op0=mybir.AluOpType.mult)
                        else:
                            tmp = s2pool.tile([P, P], f32, tag="tmpb")
                            nc.vector.tensor_scalar(
                                out=tmp[:], in0=sel4[:, tt, :],
                                scalar1=a_sb[:, tsc:tsc + 1], scalar2=None,
                                op0=mybir.AluOpType.mult)
                            nc.vector.tensor_tensor(
                                out=tmp[:], in0=tmp[:], in1=beta_sb[:],
                                op=mybir.AluOpType.add)
                            nc.scalar.activation(
                                out=out_sb[:, tsc, :], in_=tmp[:],
                                func=mybir.ActivationFunctionType.Relu)

                out_view = out[e0s: e0s + 32 * P, :].rearrange(
                    "(t p) d -> p t d", p=P)
                nc.sync.dma_start(out=out_view, in_=out_sb[:])

    nc.finalize()
    return nc


# ----------------------------------------------------------------------------
# driver
# ----------------------------------------------------------------------------


def _prep_inputs(p: Plan, x, h, snorm_n, W1, W2, ln_gamma, ln_beta):
    D = P
    use_gamma = not np.allclose(ln_gamma, 1.0)
    use_beta = not np.allclose(ln_beta, 0.0)

    x_bf = np.zeros((p.n_pad, D), dtype=BF16)
    x_bf[:p.n_nodes] = np.asarray(x).astype(BF16)

    common = {
        "x_t": np.ascontiguousarray(x_bf.T),
        "ident": np.eye(P, dtype=np.float32).astype(BF16),
        "w1aT": np.ascontiguousarray(W1[:, :D].T).astype(BF16),
        "w1bT": np.ascontiguousarray(W1[:, D:2 * D].T).astype(BF16),
        "w1cT": np.ascontiguousarray(W1[:, 2 * D:3 * D].T).astype(BF16),
        "w2T": np.ascontiguousarray(W2.T).astype(BF16),
        "ones_row": np.ones((1, P), dtype=BF16),
        "iota_col": np.arange(P, dtype=np.float32).reshape(P, 1),
        "iota_row": np.tile(np.arange(P, dtype=np.float32), (P, 1)).astype(BF16),
    }
    if use_gamma:
        common["gamma_b"] = np.tile(np.asarray(ln_gamma, np.float32), (P, 1))
    if use_beta:
        common["beta_b"] = np.tile(np.asarray(ln_beta, np.float32), (P, 1))

    in_maps, slots2_all = [], []
    for c in range(p.nc):
        m, slots2 = p.core_inputs(c, x_bf, h, snorm_n)
        m.update(common)
        in_maps.append(m)
        slots2_all.append(slots2)
    return in_maps, slots2_all, use_gamma, use_beta


def run(x, h, snorm_n, W1, W2, ln_gamma, ln_beta, src, dst, n_cores=8,
        trace=False):
    n_nodes, n_edges = x.shape[0], h.shape[0]
    p = Plan(n_nodes, n_edges, src, dst, nc=n_cores)
    in_maps, slots2_all, use_gamma, use_beta = _prep_inputs(
        p, x, h, snorm_n, W1, W2, ln_gamma, ln_beta)
    nc = build_program(p, use_gamma, use_beta)
    res = run_bass_kernel_spmd(nc, in_maps, core_ids=list(range(n_cores)),
                               trace=trace)
    out = np.empty((n_edges, P), dtype=np.float32)
    for c in range(n_cores):
        o = res.results[c]["out"]
        s = slots2_all[c]
        real = s >= 0
        out[s[real]] = o[real].astype(np.float32)
    return out, res


def kernel(x, h, snorm_n, snorm_e, W1, W2, ln_gamma, ln_beta, src, dst):
    out, _ = run(np.asarray(x), np.asarray(h), np.asarray(snorm_n),
                 np.asarray(W1), np.asarray(W2), np.asarray(ln_gamma),
                 np.asarray(ln_beta), np.asarray(src), np.asarray(dst))
    return out
